# revision 54
# baseline (speedup 1.0000x reference)
"""Expert-parallel MoE kernel for Trainium2 (8 NeuronCores).

Strategy (hardcoded for B=4, S=2048, D=768, H=3072, E=8, K=2, cap_factor=1.5):
  - Host: router (x @ Wr, softmax, top-2, capacity-limited keep in flat order),
    then dispatch: gather each expert's kept tokens (<= capacity 1536) into a
    dense per-expert buffer. This is the "all-to-all dispatch" done at
    shard-time on the host.
  - Device (SPMD, one expert per core): dense fused FFN over the expert's
    token buffer: y = gelu(x @ w1 + b1) @ w2 + b2. bf16 matmuls with fp32
    accumulate, except the last 8 of mm2's 24 contraction chunks, which run
    as 4 DoubleRow fp8(e4m3) matmuls (2x contraction per instruction,
    measured at the same 215.8ns cadence as one bf16 matmul -> saves
    4*3.9us of tensor-engine time). w2 is pre-scaled by 256 on the host so
    fp8 and bf16 terms accumulate at a consistent PSUM scale; the final
    bias activation applies scale=1/256. End-to-end maxrel vs the fp32
    reference is 1.940e-2 (deterministic, HW-verified), under the 2e-2
    gate.
  - Host: combine: out[token] += combine_weight * y  (scatter-add, unshard).

Self-contained: only needs numpy/ml_dtypes/concourse (+ axon jax devices).
"""
import os
import numpy as np
import ml_dtypes

B, S, D, H, E, TOPK = 4, 2048, 768, 3072, 8, 2
N_TOK = B * S
CAP = int((N_TOK / E) * 1.5)  # 1536
P = 128
TB = 512                      # token block (matmul free dim)
NBLK = CAP // TB              # 3
KO = D // P                   # 6  (d-chunks)
MH = H // P                   # 24 (h-chunks)
MH16 = 16                     # mm2 contraction chunks done in bf16
NPR = (MH - MH16) // 2        # 4 DoubleRow fp8 pairs (kh 16..23)
W2S = 256.0                   # host pre-scale on w2 (undone by act scale)
NCORES = 8

_CACHE = {}


def _ensure_ntff_hook_importable():
    """concourse.bass_utils' trace path does `from antenv.axon_hooks import
    get_axon_ntff_profile_hook`, which doesn't exist on slim axon images. If
    it's missing, register a stub so tracing degrades gracefully instead of
    crashing; when the axon .so with NRT-profile symbols is present, provide
    a working hook so NTFF profiling (HW exec time) works too."""
    import sys
    import types
    try:
        import antenv.axon_hooks  # noqa: F401
        return
    except ImportError:
        pass

    hook = None
    try:
        import contextlib
        import ctypes
        lib = ctypes.CDLL("/opt/axon/libaxon_pjrt.so")
        lib.axon_start_nrt_profile.argtypes = [
            ctypes.POINTER(ctypes.c_int64), ctypes.c_size_t]
        lib.axon_start_nrt_profile.restype = ctypes.c_int64
        lib.axon_stop_nrt_profile.argtypes = [ctypes.c_char_p]
        lib.axon_stop_nrt_profile.restype = ctypes.c_int64

        @contextlib.contextmanager
        def _hook(output_dir, device_ids):
            import jax
            jax.devices()
            if device_ids:
                ids = (ctypes.c_int64 * len(device_ids))(*device_ids)
                rc = lib.axon_start_nrt_profile(ids, len(device_ids))
            else:
                rc = lib.axon_start_nrt_profile(None, 0)
            if rc != 0:
                raise RuntimeError(f"axon_start_nrt_profile rc={rc}")
            try:
                yield
            finally:
                lib.axon_stop_nrt_profile(str(output_dir).encode())

        hook = _hook
    except Exception:
        hook = None

    mod = types.ModuleType("antenv.axon_hooks")
    mod.get_axon_ntff_profile_hook = lambda: hook
    mod.set_axon_ntff_profile_hook = lambda h: None
    sys.modules["antenv.axon_hooks"] = mod


def _build_nc():
    """Build + compile the per-core Bass program (identical on all 8 cores)."""
    from contextlib import ExitStack
    import concourse.mybir as mybir
    import concourse.tile as tile
    from concourse import bacc

    nc = bacc.Bacc("TRN2", target_bir_lowering=False, debug=False,
                   num_devices=NCORES)
    f32, bf16 = mybir.dt.float32, mybir.dt.bfloat16
    fp8 = mybir.dt.float8e4
    DR = mybir.MatmulPerfMode.DoubleRow

    # Layouts (host pre-arranged so every DMA is contiguous):
    #  pre0[pi, ko, 0:TB]   = x_e[t, ko*P + pi] for blk0; [TB:TB+P] = w1 mh=0
    #                         (one contiguous DMA; finer chunking pays a
    #                          ~2-3us jittery HWDGE completion receipt per
    #                          piece, and any PE gap >3.4us re-throttles HAM)
    #  xeT [pi, blk, ko, t] = x_e[blk*TB + t, ko*P + pi]   (blk 1..2 only)
    #  w1  [pi, mh, ko, hi] = w1_e[ko*P + pi, mh*P + hi]   (lhsT tiles, mm1)
    #  b1  [pi, mh] = b1_e[mh*P + pi],  b2[pi, d] = b2_e[d*P + pi]
    #  w2b [pi, d, kh, di]  = 256*w2_e[kh*P + pi, d*P + di], kh < 18 (bf16)
    #  w28 [pi, d, pr, j, di] = 256*w2_e[(18+2pr+j)*P + pi, d*P + di] (fp8)
    #  out [pi, blk, ko, t] = y_e[blk*TB + t, ko*P + pi]
    pre0 = nc.dram_tensor("pre0", [P, KO, TB + P], bf16,
                          kind="ExternalInput").ap()
    xeT = nc.dram_tensor("xeT", [P, NBLK, KO, TB], bf16, kind="ExternalInput").ap()
    w1 = nc.dram_tensor("w1", [P, MH, KO, P], bf16, kind="ExternalInput").ap()
    b1 = nc.dram_tensor("b1", [P, MH], f32, kind="ExternalInput").ap()
    w2b = nc.dram_tensor("w2b", [P, KO, MH16, P], bf16, kind="ExternalInput").ap()
    w28 = nc.dram_tensor("w28", [P, KO, NPR, 2, P], fp8, kind="ExternalInput").ap()
    b2 = nc.dram_tensor("b2", [P, KO], f32, kind="ExternalInput").ap()
    out = nc.dram_tensor("out", [P, NBLK, KO, TB], bf16,
                         kind="ExternalOutput").ap()

    GELU = mybir.ActivationFunctionType.Gelu
    IDENT = mybir.ActivationFunctionType.Identity

    with tile.TileContext(nc) as tc, ExitStack() as ctx:
        consts = ctx.enter_context(tc.tile_pool(name="consts", bufs=1))
        hpool = ctx.enter_context(tc.tile_pool(name="hpool", bufs=2))
        ypool = ctx.enter_context(tc.tile_pool(name="ypool", bufs=2))
        ps1 = ctx.enter_context(tc.tile_pool(name="ps1", bufs=4, space="PSUM"))
        ps2 = ctx.enter_context(tc.tile_pool(name="ps2", bufs=1, space="PSUM"))
        psw = ctx.enter_context(tc.tile_pool(name="psw", bufs=1, space="PSUM"))

        # PE warm-up: a short accumulation group of matmuls on a zeroed tile
        # starts the HAM activity window while the first input DMA is in
        # flight. Sized to end as pre0 slice 0 lands (~0.9us after the PE
        # queue frees up) — any longer would delay the first real matmul.
        # NOTE: the warm-up group MUST own a dedicated PSUM bank — sharing a
        # pool slot with real accumulation groups hard-faults the device
        # (NRT_EXEC_UNIT_UNRECOVERABLE), reproduced twice.
        warm = consts.tile([P, TB], bf16)
        nc.vector.memset(warm[:], 0.0)
        wps = psw.tile([P, TB], f32)
        # Sized to bridge from PE-queue-free (~7.5us) until pre0's DMA
        # completion receipt (~12.5us). Starting real matmuls earlier on
        # partial data was tried and loses: per-piece completion receipts
        # are jittery (+-1.5us), and a single PE-idle gap >3.4us mid-start
        # re-throttles the HAM clock gate back to 1.2GHz.
        NWARM = 20
        for i in range(NWARM):
            nc.tensor.matmul(wps[:, :256], warm[:, :P], warm[:, :256],
                             start=(i == 0), stop=(i == NWARM - 1))

        pre0_sb = consts.tile([P, KO, TB + P], bf16)
        w1_sb = consts.tile([P, MH, KO, P], bf16)
        xe_sb = consts.tile([P, NBLK, KO, TB], bf16)
        b1_sb = consts.tile([P, MH], f32)
        b2_sb = consts.tile([P, KO], f32)
        w2b_sb = consts.tile([P, KO, MH16, P], bf16)
        w28_sb = consts.tile([P, KO, NPR, 2, P], fp8)

        # Input DMAs ride the two HWDGE rings (sync + scalar), each FIFO in
        # emission order. Emit in exact consumption order: the six pre0
        # slices (x blk0 chunk + w1 mh=0 tile each) alternate rings so issue
        # (~0.67us per dma_start) and transfer overlap — the first real
        # matmul only waits for slice 0. The w1 ramp is sized so chunk k
        # lands just before the mh group that consumes it.
        # pre0 rides the sync ring as one contiguous DMA, with the mh=1
        # weight chunk right behind it (receipt ~0.5us later, covered by
        # mh=0's 1.3us of matmuls); b1 rides the scalar ring in parallel
        # so the first gelu never stalls behind it.
        nc.sync.dma_start(pre0_sb[:], pre0)
        nc.scalar.dma_start(b1_sb[:], b1)
        for lo, hi in ((1, 2), (2, 4), (4, 8), (8, 16), (16, 24)):
            nc.sync.dma_start(w1_sb[:, lo:hi], w1[:, lo:hi])
        nc.sync.dma_start(b2_sb[:], b2)
        nc.sync.dma_start(w2b_sb[:], w2b)
        nc.sync.dma_start(w28_sb[:], w28)
        nc.sync.dma_start(xe_sb[:, 1:], xeT[:, 1:])

        for blk in range(NBLK):
            # mm1: hT[h, t] = gelu(sum_ko w1[ko,:].T @ x[ko,:] + b1)
            # kh chunks < 18 stored bf16; chunks 18..23 stored fp8 as
            # DoubleRow pairs for the mm2 sprinkle.
            hT = hpool.tile([P, MH16, TB], bf16)
            hT8 = hpool.tile([P, NPR, 2, TB], fp8)
            for mh in range(MH):
                ps = ps1.tile([P, TB], f32)
                for ko in range(KO):
                    lhsT = (pre0_sb[:, ko, TB:] if mh == 0
                            else w1_sb[:, mh, ko])
                    rhs = (pre0_sb[:, ko, :TB] if blk == 0
                           else xe_sb[:, blk, ko])
                    nc.tensor.matmul(ps[:], lhsT, rhs,
                                     start=(ko == 0), stop=(ko == KO - 1))
                if mh < MH16:
                    nc.scalar.activation(hT[:, mh], ps[:], GELU,
                                         bias=b1_sb[:, mh:mh + 1])
                else:
                    pr, j = divmod(mh - MH16, 2)
                    nc.scalar.activation(hT8[:, pr, j], ps[:], GELU,
                                         bias=b1_sb[:, mh:mh + 1])
            # mm2: yT[d, t] = (sum_kh 256*w2[kh,:].T @ h[kh,:]) / 256 + b2
            yT = ypool.tile([P, KO, TB], bf16)
            # Each bf16<->DoubleRow mode transition on the tensor engine
            # costs a ~190ns bubble, so the DR matmuls are clustered: the
            # bf16 parts of three d-chunks run back-to-back, then all
            # three d-chunks' DR matmuls (one transition pair per THREE
            # d-chunks). All three PSUM groups stay open across the
            # cluster (3 single-buf ps2 tags).
            for d0 in range(0, KO, 3):
                pca = ps2.tile([P, TB], f32)
                pcb = ps2.tile([P, TB], f32)
                pcc = ps2.tile([P, TB], f32)
                trio = ((pca, d0), (pcb, d0 + 1), (pcc, d0 + 2))
                for ps, d in trio:
                    for kh in range(MH16):
                        nc.tensor.matmul(ps[:], w2b_sb[:, d, kh],
                                         hT[:, kh], start=(kh == 0),
                                         stop=False)
                last_trio = blk == NBLK - 1 and d0 == KO - 3
                for ps, d in trio:
                    for pr in range(NPR):
                        nc.tensor.matmul(ps[:], w28_sb[:, d, pr],
                                         hT8[:, pr], start=False,
                                         stop=(pr == NPR - 1),
                                         perf_mode=DR)
                for ps, d in trio:
                    if last_trio and d >= KO - 2:
                        # Tail: both final d-chunks' bias-activations run
                        # split across the scalar and vector engines, and
                        # the output stores issue on both HWDGE rings, so
                        # the post-stream exposure is one short act + one
                        # DMA receipt.
                        hh = TB // 2
                        nc.scalar.activation(yT[:, d, :hh], ps[:, :hh],
                                             IDENT, bias=b2_sb[:, d:d + 1],
                                             scale=1.0 / W2S)
                        nc.vector.tensor_scalar(
                            yT[:, d, hh:], ps[:, hh:], 1.0 / W2S,
                            b2_sb[:, d:d + 1], mybir.AluOpType.mult,
                            mybir.AluOpType.add)
                        nc.sync.dma_start(out[:, blk, d, :hh],
                                          yT[:, d, :hh])
                        nc.scalar.dma_start(out[:, blk, d, hh:],
                                            yT[:, d, hh:])
                    else:
                        nc.scalar.activation(yT[:, d], ps[:], IDENT,
                                             bias=b2_sb[:, d:d + 1],
                                             scale=1.0 / W2S)
                        nc.sync.dma_start(out[:, blk, d], yT[:, d])

    nc.compile()
    return nc


def _route(x_flat, Wr):
    """Reproduce the reference router exactly: softmax -> top-2 -> renormalize
    -> capacity-limited keep in flat (token-major, k-inner) order."""
    logits = x_flat @ Wr
    m = logits.max(-1, keepdims=True)
    ex = np.exp(logits - m)
    probs = ex / ex.sum(-1, keepdims=True)
    n = np.arange(N_TOK)
    i1 = probs.argmax(-1)
    p1 = probs[n, i1]
    probs2 = probs.copy()
    probs2[n, i1] = -np.inf
    i2 = probs2.argmax(-1)
    p2 = probs[n, i2]
    s = p1 + p2
    e_flat = np.stack([i1, i2], -1).reshape(-1)          # [2N] expert ids
    p_flat = np.stack([p1 / s, p2 / s], -1).reshape(-1)  # [2N] combine weights
    order = np.argsort(e_flat, kind="stable")            # flat order per expert
    sorted_e = e_flat[order]
    starts = np.searchsorted(sorted_e, np.arange(E))
    ends = np.searchsorted(sorted_e, np.arange(E), side="right")
    toks, wgts = [], []
    for e in range(E):
        kept = order[starts[e] : min(ends[e], starts[e] + CAP)]
        toks.append(kept // TOPK)
        wgts.append(p_flat[kept].astype(np.float32))
    return toks, wgts


def kernel(x, Wr, w1, b1, w2, b2):
    _ensure_ntff_hook_importable()
    from concourse import bass_utils

    x = np.asarray(x, np.float32)
    Wr = np.asarray(Wr, np.float32)
    w1 = np.asarray(w1, np.float32)
    b1 = np.asarray(b1, np.float32)
    w2 = np.asarray(w2, np.float32)
    b2 = np.asarray(b2, np.float32)

    x_flat = x.reshape(N_TOK, D)
    toks, wgts = _route(x_flat, Wr)

    if "nc" not in _CACHE:
        _CACHE["nc"] = _build_nc()
    nc = _CACHE["nc"]

    bf = ml_dtypes.bfloat16
    f8 = ml_dtypes.float8_e4m3fn
    in_maps = []
    for e in range(E):
        cnt = len(toks[e])
        xe = np.zeros((CAP, D), np.float32)
        xe[:cnt] = x_flat[toks[e]]
        xeT = np.ascontiguousarray(
            xe.reshape(NBLK, TB, KO, P).transpose(3, 0, 2, 1)).astype(bf)
        w1r = np.ascontiguousarray(
            w1[e].reshape(KO, P, MH, P).transpose(1, 2, 0, 3)).astype(bf)
        w2s = (w2[e] * W2S).reshape(MH, P, KO, P)
        # [pi, d, kh, di] from w2s[kh, pi, d, di]
        w2r = w2s.transpose(1, 2, 0, 3)
        w2b_ = np.ascontiguousarray(w2r[:, :, :MH16]).astype(bf)
        w28_ = np.ascontiguousarray(
            np.clip(w2r[:, :, MH16:], -240.0, 240.0)
        ).reshape(P, KO, NPR, 2, P).astype(f8)
        in_maps.append({
            "pre0": np.ascontiguousarray(
                np.concatenate([xeT[:, 0], w1r[:, 0]], axis=-1)),
            "xeT": xeT,
            "w1": w1r,
            "b1": np.ascontiguousarray(b1[e].reshape(MH, P).T),
            "w2b": w2b_,
            "w28": w28_,
            "b2": np.ascontiguousarray(b2[e].reshape(KO, P).T),
        })

    trace = bool(os.environ.get("MOE_TRACE"))
    try:
        res = bass_utils.run_bass_kernel_spmd(
            nc, in_maps, core_ids=list(range(NCORES)), trace=trace)
    except Exception:
        if trace or os.environ.get("BASS_TRACE"):
            # Profiling infrastructure failure — rerun without tracing.
            os.environ["BASS_NEVER_TRACE"] = "1"
            res = bass_utils.run_bass_kernel_spmd(
                nc, in_maps, core_ids=list(range(NCORES)), trace=False)
        else:
            raise
    _CACHE["last_results"] = res

    out = np.zeros((N_TOK, D), np.float32)
    for e in range(E):
        y = res.results[e]["out"].astype(np.float32)   # [P, NBLK, KO, TB]
        y = y.transpose(1, 3, 2, 0).reshape(CAP, D)
        cnt = len(toks[e])
        # token ids are unique within one expert, so fancy-index += is safe
        out[toks[e]] += y[:cnt] * wgts[e][:, None]
    return out.reshape(B, S, D)


# revision 55
# speedup vs baseline: 1.0161x; 1.0161x over previous
"""Expert-parallel MoE kernel for Trainium2 (8 NeuronCores).

Strategy (hardcoded for B=4, S=2048, D=768, H=3072, E=8, K=2, cap_factor=1.5):
  - Host: router (x @ Wr, softmax, top-2, capacity-limited keep in flat order),
    then dispatch: gather each expert's kept tokens (<= capacity 1536) into a
    dense per-expert buffer. This is the "all-to-all dispatch" done at
    shard-time on the host.
  - Device (SPMD, one expert per core): dense fused FFN over the expert's
    token buffer: y = gelu(x @ w1 + b1) @ w2 + b2. bf16 matmuls with fp32
    accumulate, except the last 8 of mm2's 24 contraction chunks, which run
    as 4 DoubleRow fp8(e4m3) matmuls (2x contraction per instruction,
    measured at the same 215.8ns cadence as one bf16 matmul -> saves
    4*3.9us of tensor-engine time). w2 is pre-scaled by 256 on the host so
    fp8 and bf16 terms accumulate at a consistent PSUM scale; the final
    bias activation applies scale=1/256. End-to-end maxrel vs the fp32
    reference is 1.940e-2 (deterministic, HW-verified), under the 2e-2
    gate.
  - Host: combine: out[token] += combine_weight * y  (scatter-add, unshard).

Self-contained: only needs numpy/ml_dtypes/concourse (+ axon jax devices).
"""
import os
import numpy as np
import ml_dtypes

B, S, D, H, E, TOPK = 4, 2048, 768, 3072, 8, 2
N_TOK = B * S
CAP = int((N_TOK / E) * 1.5)  # 1536
P = 128
TB = 512                      # token block (matmul free dim)
NBLK = CAP // TB              # 3
KO = D // P                   # 6  (d-chunks)
MH = H // P                   # 24 (h-chunks)
MH16 = 16                     # mm2 contraction chunks done in bf16
NPR = (MH - MH16) // 2        # 4 DoubleRow fp8 pairs (kh 16..23)
W2S = 256.0                   # host pre-scale on w2 (undone by act scale)
NCORES = 8

_CACHE = {}


def _ensure_ntff_hook_importable():
    """concourse.bass_utils' trace path does `from antenv.axon_hooks import
    get_axon_ntff_profile_hook`, which doesn't exist on slim axon images. If
    it's missing, register a stub so tracing degrades gracefully instead of
    crashing; when the axon .so with NRT-profile symbols is present, provide
    a working hook so NTFF profiling (HW exec time) works too."""
    import sys
    import types
    try:
        import antenv.axon_hooks  # noqa: F401
        return
    except ImportError:
        pass

    hook = None
    try:
        import contextlib
        import ctypes
        lib = ctypes.CDLL("/opt/axon/libaxon_pjrt.so")
        lib.axon_start_nrt_profile.argtypes = [
            ctypes.POINTER(ctypes.c_int64), ctypes.c_size_t]
        lib.axon_start_nrt_profile.restype = ctypes.c_int64
        lib.axon_stop_nrt_profile.argtypes = [ctypes.c_char_p]
        lib.axon_stop_nrt_profile.restype = ctypes.c_int64

        @contextlib.contextmanager
        def _hook(output_dir, device_ids):
            import jax
            jax.devices()
            if device_ids:
                ids = (ctypes.c_int64 * len(device_ids))(*device_ids)
                rc = lib.axon_start_nrt_profile(ids, len(device_ids))
            else:
                rc = lib.axon_start_nrt_profile(None, 0)
            if rc != 0:
                raise RuntimeError(f"axon_start_nrt_profile rc={rc}")
            try:
                yield
            finally:
                lib.axon_stop_nrt_profile(str(output_dir).encode())

        hook = _hook
    except Exception:
        hook = None

    mod = types.ModuleType("antenv.axon_hooks")
    mod.get_axon_ntff_profile_hook = lambda: hook
    mod.set_axon_ntff_profile_hook = lambda h: None
    sys.modules["antenv.axon_hooks"] = mod


def _build_nc():
    """Build + compile the per-core Bass program (identical on all 8 cores)."""
    from contextlib import ExitStack
    import concourse.mybir as mybir
    import concourse.tile as tile
    from concourse import bacc

    nc = bacc.Bacc("TRN2", target_bir_lowering=False, debug=False,
                   num_devices=NCORES)
    f32, bf16 = mybir.dt.float32, mybir.dt.bfloat16
    fp8 = mybir.dt.float8e4
    DR = mybir.MatmulPerfMode.DoubleRow

    # Layouts (host pre-arranged so every DMA is contiguous):
    #  pre0[pi, ko, 0:TB]   = x_e[t, ko*P + pi] for blk0; [TB:TB+P] = w1 mh=0
    #                         (one contiguous DMA; finer chunking pays a
    #                          ~2-3us jittery HWDGE completion receipt per
    #                          piece, and any PE gap >3.4us re-throttles HAM)
    #  xeT [pi, blk, ko, t] = x_e[blk*TB + t, ko*P + pi]   (blk 1..2 only)
    #  w1  [pi, mh, ko, hi] = w1_e[ko*P + pi, mh*P + hi]   (lhsT tiles, mm1)
    #  b1  [pi, mh] = b1_e[mh*P + pi],  b2[pi, d] = b2_e[d*P + pi]
    #  w2b [pi, d, kh, di]  = 256*w2_e[kh*P + pi, d*P + di], kh < 18 (bf16)
    #  w28 [pi, d, pr, j, di] = 256*w2_e[(18+2pr+j)*P + pi, d*P + di] (fp8)
    #  out [pi, blk, ko, t] = y_e[blk*TB + t, ko*P + pi]
    pre0 = nc.dram_tensor("pre0", [P, KO, TB + P], bf16,
                          kind="ExternalInput").ap()
    xeT = nc.dram_tensor("xeT", [P, NBLK, KO, TB], bf16, kind="ExternalInput").ap()
    w1 = nc.dram_tensor("w1", [P, MH, KO, P], bf16, kind="ExternalInput").ap()
    b1 = nc.dram_tensor("b1", [P, MH], f32, kind="ExternalInput").ap()
    w2b = nc.dram_tensor("w2b", [P, KO, MH16, P], bf16, kind="ExternalInput").ap()
    w28 = nc.dram_tensor("w28", [P, KO, NPR, 2, P], fp8, kind="ExternalInput").ap()
    b2 = nc.dram_tensor("b2", [P, KO], f32, kind="ExternalInput").ap()
    out = nc.dram_tensor("out", [P, NBLK, KO, TB], bf16,
                         kind="ExternalOutput").ap()

    GELU = mybir.ActivationFunctionType.Gelu
    IDENT = mybir.ActivationFunctionType.Identity

    with tile.TileContext(nc) as tc, ExitStack() as ctx:
        consts = ctx.enter_context(tc.tile_pool(name="consts", bufs=1))
        hpool = ctx.enter_context(tc.tile_pool(name="hpool", bufs=2))
        ypool = ctx.enter_context(tc.tile_pool(name="ypool", bufs=2))
        ps1 = ctx.enter_context(tc.tile_pool(name="ps1", bufs=4, space="PSUM"))
        ps2 = ctx.enter_context(tc.tile_pool(name="ps2", bufs=1, space="PSUM"))
        psw = ctx.enter_context(tc.tile_pool(name="psw", bufs=1, space="PSUM"))

        # PE warm-up: a short accumulation group of matmuls on a zeroed tile
        # starts the HAM activity window while the first input DMA is in
        # flight. Sized to end as pre0 slice 0 lands (~0.9us after the PE
        # queue frees up) — any longer would delay the first real matmul.
        # NOTE: the warm-up group MUST own a dedicated PSUM bank — sharing a
        # pool slot with real accumulation groups hard-faults the device
        # (NRT_EXEC_UNIT_UNRECOVERABLE), reproduced twice.
        warm = consts.tile([P, TB], bf16)
        nc.vector.memset(warm[:], 0.0)
        wps = psw.tile([P, TB], f32)
        # Sized to bridge from PE-queue-free (~7.5us) until pre0's DMA
        # completion receipt (~12.5us). Starting real matmuls earlier on
        # partial data was tried and loses: per-piece completion receipts
        # are jittery (+-1.5us), and a single PE-idle gap >3.4us mid-start
        # re-throttles the HAM clock gate back to 1.2GHz.
        # 24 x 213ns ends ~13.1us, just past the typical pre0 receipt
        # (12.4-13.1us, jitter to 15). Shorter warmup measured WORSE: a
        # >2us PE-idle gap before the real matmuls lets the HAM MID window
        # re-throttle the PE to 1.2GHz (costs ~3us).
        NWARM = 24
        for i in range(NWARM):
            nc.tensor.matmul(wps[:, :256], warm[:, :P], warm[:, :256],
                             start=(i == 0), stop=(i == NWARM - 1))

        pre0_sb = consts.tile([P, KO, TB + P], bf16)
        w1_sb = consts.tile([P, MH, KO, P], bf16)
        xe_sb = consts.tile([P, NBLK, KO, TB], bf16)
        b1_sb = consts.tile([P, MH], f32)
        b2_sb = consts.tile([P, KO], f32)
        w2b_sb = consts.tile([P, KO, MH16, P], bf16)
        w28_sb = consts.tile([P, KO, NPR, 2, P], fp8)

        # Input DMAs ride the two HWDGE rings (sync + scalar), each FIFO in
        # emission order. Emit in exact consumption order: the six pre0
        # slices (x blk0 chunk + w1 mh=0 tile each) alternate rings so issue
        # (~0.67us per dma_start) and transfer overlap — the first real
        # matmul only waits for slice 0. The w1 ramp is sized so chunk k
        # lands just before the mh group that consumes it.
        # pre0 rides the sync ring as one contiguous DMA, with the mh=1
        # weight chunk right behind it (receipt ~0.5us later, covered by
        # mh=0's 1.3us of matmuls); b1 rides the scalar ring in parallel
        # so the first gelu never stalls behind it.
        nc.sync.dma_start(pre0_sb[:], pre0)
        nc.scalar.dma_start(b1_sb[:], b1)
        for lo, hi in ((1, 2), (2, 4), (4, 8), (8, 16), (16, 24)):
            nc.sync.dma_start(w1_sb[:, lo:hi], w1[:, lo:hi])
        nc.sync.dma_start(b2_sb[:], b2)
        nc.sync.dma_start(w2b_sb[:], w2b)
        nc.sync.dma_start(w28_sb[:], w28)
        nc.sync.dma_start(xe_sb[:, 1:], xeT[:, 1:])

        for blk in range(NBLK):
            # mm1: hT[h, t] = gelu(sum_ko w1[ko,:].T @ x[ko,:] + b1)
            # kh chunks < 18 stored bf16; chunks 18..23 stored fp8 as
            # DoubleRow pairs for the mm2 sprinkle.
            hT = hpool.tile([P, MH16, TB], bf16)
            hT8 = hpool.tile([P, NPR, 2, TB], fp8)
            for mh in range(MH):
                ps = ps1.tile([P, TB], f32)
                for ko in range(KO):
                    lhsT = (pre0_sb[:, ko, TB:] if mh == 0
                            else w1_sb[:, mh, ko])
                    rhs = (pre0_sb[:, ko, :TB] if blk == 0
                           else xe_sb[:, blk, ko])
                    nc.tensor.matmul(ps[:], lhsT, rhs,
                                     start=(ko == 0), stop=(ko == KO - 1))
                if mh < MH16:
                    nc.scalar.activation(hT[:, mh], ps[:], GELU,
                                         bias=b1_sb[:, mh:mh + 1])
                else:
                    pr, j = divmod(mh - MH16, 2)
                    nc.scalar.activation(hT8[:, pr, j], ps[:], GELU,
                                         bias=b1_sb[:, mh:mh + 1])
            # mm2: yT[d, t] = (sum_kh 256*w2[kh,:].T @ h[kh,:]) / 256 + b2
            yT = ypool.tile([P, KO, TB], bf16)
            # Each bf16<->DoubleRow mode transition on the tensor engine
            # costs a ~190ns bubble, so the DR matmuls are clustered: the
            # bf16 parts of three d-chunks run back-to-back, then all
            # three d-chunks' DR matmuls (one transition pair per THREE
            # d-chunks). All three PSUM groups stay open across the
            # cluster (3 single-buf ps2 tags).
            for d0 in range(0, KO, 3):
                pca = ps2.tile([P, TB], f32)
                pcb = ps2.tile([P, TB], f32)
                pcc = ps2.tile([P, TB], f32)
                trio = ((pca, d0), (pcb, d0 + 1), (pcc, d0 + 2))
                for ps, d in trio:
                    for kh in range(MH16):
                        nc.tensor.matmul(ps[:], w2b_sb[:, d, kh],
                                         hT[:, kh], start=(kh == 0),
                                         stop=False)
                last_trio = blk == NBLK - 1 and d0 == KO - 3
                for ps, d in trio:
                    for pr in range(NPR):
                        nc.tensor.matmul(ps[:], w28_sb[:, d, pr],
                                         hT8[:, pr], start=False,
                                         stop=(pr == NPR - 1),
                                         perf_mode=DR)
                for ps, d in trio:
                    if last_trio and d >= KO - 2:
                        # Tail: both final d-chunks' bias-activations run
                        # split across the scalar and vector engines, and
                        # the output stores issue on both HWDGE rings, so
                        # the post-stream exposure is one short act + one
                        # DMA receipt.
                        hh = TB // 2
                        nc.scalar.activation(yT[:, d, :hh], ps[:, :hh],
                                             IDENT, bias=b2_sb[:, d:d + 1],
                                             scale=1.0 / W2S)
                        nc.vector.tensor_scalar(
                            yT[:, d, hh:], ps[:, hh:], 1.0 / W2S,
                            b2_sb[:, d:d + 1], mybir.AluOpType.mult,
                            mybir.AluOpType.add)
                        nc.sync.dma_start(out[:, blk, d, :hh],
                                          yT[:, d, :hh])
                        nc.scalar.dma_start(out[:, blk, d, hh:],
                                            yT[:, d, hh:])
                    else:
                        nc.scalar.activation(yT[:, d], ps[:], IDENT,
                                             bias=b2_sb[:, d:d + 1],
                                             scale=1.0 / W2S)
                        nc.sync.dma_start(out[:, blk, d], yT[:, d])

    nc.compile()
    return nc


def _route(x_flat, Wr):
    """Reproduce the reference router exactly: softmax -> top-2 -> renormalize
    -> capacity-limited keep in flat (token-major, k-inner) order."""
    logits = x_flat @ Wr
    m = logits.max(-1, keepdims=True)
    ex = np.exp(logits - m)
    probs = ex / ex.sum(-1, keepdims=True)
    n = np.arange(N_TOK)
    i1 = probs.argmax(-1)
    p1 = probs[n, i1]
    probs2 = probs.copy()
    probs2[n, i1] = -np.inf
    i2 = probs2.argmax(-1)
    p2 = probs[n, i2]
    s = p1 + p2
    e_flat = np.stack([i1, i2], -1).reshape(-1)          # [2N] expert ids
    p_flat = np.stack([p1 / s, p2 / s], -1).reshape(-1)  # [2N] combine weights
    order = np.argsort(e_flat, kind="stable")            # flat order per expert
    sorted_e = e_flat[order]
    starts = np.searchsorted(sorted_e, np.arange(E))
    ends = np.searchsorted(sorted_e, np.arange(E), side="right")
    toks, wgts = [], []
    for e in range(E):
        kept = order[starts[e] : min(ends[e], starts[e] + CAP)]
        toks.append(kept // TOPK)
        wgts.append(p_flat[kept].astype(np.float32))
    return toks, wgts


def kernel(x, Wr, w1, b1, w2, b2):
    _ensure_ntff_hook_importable()
    from concourse import bass_utils

    x = np.asarray(x, np.float32)
    Wr = np.asarray(Wr, np.float32)
    w1 = np.asarray(w1, np.float32)
    b1 = np.asarray(b1, np.float32)
    w2 = np.asarray(w2, np.float32)
    b2 = np.asarray(b2, np.float32)

    x_flat = x.reshape(N_TOK, D)
    toks, wgts = _route(x_flat, Wr)

    if "nc" not in _CACHE:
        _CACHE["nc"] = _build_nc()
    nc = _CACHE["nc"]

    bf = ml_dtypes.bfloat16
    f8 = ml_dtypes.float8_e4m3fn
    in_maps = []
    for e in range(E):
        cnt = len(toks[e])
        xe = np.zeros((CAP, D), np.float32)
        xe[:cnt] = x_flat[toks[e]]
        xeT = np.ascontiguousarray(
            xe.reshape(NBLK, TB, KO, P).transpose(3, 0, 2, 1)).astype(bf)
        w1r = np.ascontiguousarray(
            w1[e].reshape(KO, P, MH, P).transpose(1, 2, 0, 3)).astype(bf)
        w2s = (w2[e] * W2S).reshape(MH, P, KO, P)
        # [pi, d, kh, di] from w2s[kh, pi, d, di]
        w2r = w2s.transpose(1, 2, 0, 3)
        w2b_ = np.ascontiguousarray(w2r[:, :, :MH16]).astype(bf)
        w28_ = np.ascontiguousarray(
            np.clip(w2r[:, :, MH16:], -240.0, 240.0)
        ).reshape(P, KO, NPR, 2, P).astype(f8)
        in_maps.append({
            "pre0": np.ascontiguousarray(
                np.concatenate([xeT[:, 0], w1r[:, 0]], axis=-1)),
            "xeT": xeT,
            "w1": w1r,
            "b1": np.ascontiguousarray(b1[e].reshape(MH, P).T),
            "w2b": w2b_,
            "w28": w28_,
            "b2": np.ascontiguousarray(b2[e].reshape(KO, P).T),
        })

    trace = bool(os.environ.get("MOE_TRACE"))
    try:
        res = bass_utils.run_bass_kernel_spmd(
            nc, in_maps, core_ids=list(range(NCORES)), trace=trace)
    except Exception:
        if trace or os.environ.get("BASS_TRACE"):
            # Profiling infrastructure failure — rerun without tracing.
            os.environ["BASS_NEVER_TRACE"] = "1"
            res = bass_utils.run_bass_kernel_spmd(
                nc, in_maps, core_ids=list(range(NCORES)), trace=False)
        else:
            raise
    _CACHE["last_results"] = res

    out = np.zeros((N_TOK, D), np.float32)
    for e in range(E):
        y = res.results[e]["out"].astype(np.float32)   # [P, NBLK, KO, TB]
        y = y.transpose(1, 3, 2, 0).reshape(CAP, D)
        cnt = len(toks[e])
        # token ids are unique within one expert, so fancy-index += is safe
        out[toks[e]] += y[:cnt] * wgts[e][:, None]
    return out.reshape(B, S, D)


# revision 57
# speedup vs baseline: 1.0183x; 1.0022x over previous
"""Expert-parallel MoE kernel for Trainium2 (8 NeuronCores).

Strategy (hardcoded for B=4, S=2048, D=768, H=3072, E=8, K=2, cap_factor=1.5):
  - Host: router (x @ Wr, softmax, top-2, capacity-limited keep in flat order),
    then dispatch: gather each expert's kept tokens (<= capacity 1536) into a
    dense per-expert buffer. This is the "all-to-all dispatch" done at
    shard-time on the host.
  - Device (SPMD, one expert per core): dense fused FFN over the expert's
    token buffer: y = gelu(x @ w1 + b1) @ w2 + b2. bf16 matmuls with fp32
    accumulate, except the last 8 of mm2's 24 contraction chunks, which run
    as 4 DoubleRow fp8(e4m3) matmuls (2x contraction per instruction,
    measured at the same 215.8ns cadence as one bf16 matmul -> saves
    4*3.9us of tensor-engine time). w2 is pre-scaled by 256 on the host so
    fp8 and bf16 terms accumulate at a consistent PSUM scale; the final
    bias activation applies scale=1/256. End-to-end maxrel vs the fp32
    reference is 1.940e-2 (deterministic, HW-verified), under the 2e-2
    gate.
  - Host: combine: out[token] += combine_weight * y  (scatter-add, unshard).

Self-contained: only needs numpy/ml_dtypes/concourse (+ axon jax devices).
"""
import os
import numpy as np
import ml_dtypes

B, S, D, H, E, TOPK = 4, 2048, 768, 3072, 8, 2
N_TOK = B * S
CAP = int((N_TOK / E) * 1.5)  # 1536
P = 128
TB = 512                      # token block (matmul free dim)
NBLK = CAP // TB              # 3
KO = D // P                   # 6  (d-chunks)
MH = H // P                   # 24 (h-chunks)
MH16 = 16                     # mm2 contraction chunks done in bf16
NPR = (MH - MH16) // 2        # 4 DoubleRow fp8 pairs (kh 16..23)
W2S = 256.0                   # host pre-scale on w2 (undone by act scale)
NCORES = 8

_CACHE = {}


def _ensure_ntff_hook_importable():
    """concourse.bass_utils' trace path does `from antenv.axon_hooks import
    get_axon_ntff_profile_hook`, which doesn't exist on slim axon images. If
    it's missing, register a stub so tracing degrades gracefully instead of
    crashing; when the axon .so with NRT-profile symbols is present, provide
    a working hook so NTFF profiling (HW exec time) works too."""
    import sys
    import types
    try:
        import antenv.axon_hooks  # noqa: F401
        return
    except ImportError:
        pass

    hook = None
    try:
        import contextlib
        import ctypes
        lib = ctypes.CDLL("/opt/axon/libaxon_pjrt.so")
        lib.axon_start_nrt_profile.argtypes = [
            ctypes.POINTER(ctypes.c_int64), ctypes.c_size_t]
        lib.axon_start_nrt_profile.restype = ctypes.c_int64
        lib.axon_stop_nrt_profile.argtypes = [ctypes.c_char_p]
        lib.axon_stop_nrt_profile.restype = ctypes.c_int64

        @contextlib.contextmanager
        def _hook(output_dir, device_ids):
            import jax
            jax.devices()
            if device_ids:
                ids = (ctypes.c_int64 * len(device_ids))(*device_ids)
                rc = lib.axon_start_nrt_profile(ids, len(device_ids))
            else:
                rc = lib.axon_start_nrt_profile(None, 0)
            if rc != 0:
                raise RuntimeError(f"axon_start_nrt_profile rc={rc}")
            try:
                yield
            finally:
                lib.axon_stop_nrt_profile(str(output_dir).encode())

        hook = _hook
    except Exception:
        hook = None

    mod = types.ModuleType("antenv.axon_hooks")
    mod.get_axon_ntff_profile_hook = lambda: hook
    mod.set_axon_ntff_profile_hook = lambda h: None
    sys.modules["antenv.axon_hooks"] = mod


def _build_nc():
    """Build + compile the per-core Bass program (identical on all 8 cores)."""
    from contextlib import ExitStack
    import concourse.mybir as mybir
    import concourse.tile as tile
    from concourse import bacc

    nc = bacc.Bacc("TRN2", target_bir_lowering=False, debug=False,
                   num_devices=NCORES)
    f32, bf16 = mybir.dt.float32, mybir.dt.bfloat16
    fp8 = mybir.dt.float8e4
    DR = mybir.MatmulPerfMode.DoubleRow

    # Layouts (host pre-arranged so every DMA is contiguous):
    #  pre0[pi, ko, 0:TB]   = x_e[t, ko*P + pi] for blk0; [TB:TB+P] = w1 mh=0
    #                         (one contiguous DMA; finer chunking pays a
    #                          ~2-3us jittery HWDGE completion receipt per
    #                          piece, and any PE gap >3.4us re-throttles HAM)
    #  xeT [pi, blk, ko, t] = x_e[blk*TB + t, ko*P + pi]   (blk 1..2 only)
    #  w1  [pi, mh, ko, hi] = w1_e[ko*P + pi, mh*P + hi]   (lhsT tiles, mm1)
    #  b1  [pi, mh] = b1_e[mh*P + pi],  b2[pi, d] = b2_e[d*P + pi]
    #  w2b [pi, d, kh, di]  = 256*w2_e[kh*P + pi, d*P + di], kh < 18 (bf16)
    #  w28 [pi, d, pr, j, di] = 256*w2_e[(18+2pr+j)*P + pi, d*P + di] (fp8)
    #  out [pi, blk, ko, t] = y_e[blk*TB + t, ko*P + pi]
    pre0 = nc.dram_tensor("pre0", [P, KO, TB + P], bf16,
                          kind="ExternalInput").ap()
    xeT = nc.dram_tensor("xeT", [P, NBLK, KO, TB], bf16, kind="ExternalInput").ap()
    w1 = nc.dram_tensor("w1", [P, MH, KO, P], bf16, kind="ExternalInput").ap()
    b1 = nc.dram_tensor("b1", [P, MH], f32, kind="ExternalInput").ap()
    w2b = nc.dram_tensor("w2b", [P, KO, MH16, P], bf16, kind="ExternalInput").ap()
    w28 = nc.dram_tensor("w28", [P, KO, NPR, 2, P], fp8, kind="ExternalInput").ap()
    b2 = nc.dram_tensor("b2", [P, KO], f32, kind="ExternalInput").ap()
    out = nc.dram_tensor("out", [P, NBLK, KO, TB], bf16,
                         kind="ExternalOutput").ap()

    GELU = mybir.ActivationFunctionType.Gelu
    IDENT = mybir.ActivationFunctionType.Identity

    with tile.TileContext(nc) as tc, ExitStack() as ctx:
        # Only the pools needed by the warm-up and input DMAs are entered
        # up front — each pool entry emits per-queue plumbing (ordering
        # modes, drains) that would otherwise delay the first dma_start and
        # warm-up matmul by over a microsecond. The remaining pools are
        # entered after the input DMAs are already queued.
        consts = ctx.enter_context(tc.tile_pool(name="consts", bufs=1))
        psw = ctx.enter_context(tc.tile_pool(name="psw", bufs=1, space="PSUM"))

        # PE warm-up: a short accumulation group of matmuls on a zeroed tile
        # starts the HAM activity window while the first input DMA is in
        # flight. Sized to end as pre0 slice 0 lands (~0.9us after the PE
        # queue frees up) — any longer would delay the first real matmul.
        # NOTE: the warm-up group MUST own a dedicated PSUM bank — sharing a
        # pool slot with real accumulation groups hard-faults the device
        # (NRT_EXEC_UNIT_UNRECOVERABLE), reproduced twice.
        warm = consts.tile([P, TB], bf16)
        nc.vector.memset(warm[:], 0.0)
        wps = psw.tile([P, TB], f32)
        # Sized to bridge from PE-queue-free (~7.5us) until pre0's DMA
        # completion receipt (~12.5us). Starting real matmuls earlier on
        # partial data was tried and loses: per-piece completion receipts
        # are jittery (+-1.5us), and a single PE-idle gap >3.4us mid-start
        # re-throttles the HAM clock gate back to 1.2GHz.
        # 24 x 213ns ends ~13.1us, just past the typical pre0 receipt
        # (12.4-13.1us, jitter to 15). Shorter warmup measured WORSE: a
        # >2us PE-idle gap before the real matmuls lets the HAM MID window
        # re-throttle the PE to 1.2GHz (costs ~3us).
        NWARM = 24
        for i in range(NWARM):
            nc.tensor.matmul(wps[:, :256], warm[:, :P], warm[:, :256],
                             start=(i == 0), stop=(i == NWARM - 1))

        pre0_sb = consts.tile([P, KO, TB + P], bf16)
        w1_sb = consts.tile([P, MH, KO, P], bf16)
        xe_sb = consts.tile([P, NBLK, KO, TB], bf16)
        b1_sb = consts.tile([P, MH], f32)
        b2_sb = consts.tile([P, KO], f32)
        w2b_sb = consts.tile([P, KO, MH16, P], bf16)
        w28_sb = consts.tile([P, KO, NPR, 2, P], fp8)

        # Input DMAs ride the two HWDGE rings (sync + scalar), each FIFO in
        # emission order. Emit in exact consumption order: the six pre0
        # slices (x blk0 chunk + w1 mh=0 tile each) alternate rings so issue
        # (~0.67us per dma_start) and transfer overlap — the first real
        # matmul only waits for slice 0. The w1 ramp is sized so chunk k
        # lands just before the mh group that consumes it.
        # pre0 rides the sync ring as one contiguous DMA, with the mh=1
        # weight chunk right behind it (receipt ~0.5us later, covered by
        # mh=0's 1.3us of matmuls); b1 rides the scalar ring in parallel
        # so the first gelu never stalls behind it.
        nc.sync.dma_start(pre0_sb[:], pre0)
        nc.scalar.dma_start(b1_sb[:], b1)
        for lo, hi in ((1, 2), (2, 4), (4, 8), (8, 16), (16, 24)):
            nc.sync.dma_start(w1_sb[:, lo:hi], w1[:, lo:hi])
        nc.sync.dma_start(b2_sb[:], b2)
        nc.sync.dma_start(w2b_sb[:], w2b)
        nc.sync.dma_start(w28_sb[:], w28)
        nc.sync.dma_start(xe_sb[:, 1:], xeT[:, 1:])

        hpool = ctx.enter_context(tc.tile_pool(name="hpool", bufs=2))
        ypool = ctx.enter_context(tc.tile_pool(name="ypool", bufs=2))
        ps1 = ctx.enter_context(tc.tile_pool(name="ps1", bufs=4, space="PSUM"))
        ps2 = ctx.enter_context(tc.tile_pool(name="ps2", bufs=1, space="PSUM"))

        for blk in range(NBLK):
            # mm1: hT[h, t] = gelu(sum_ko w1[ko,:].T @ x[ko,:] + b1)
            # kh chunks < 18 stored bf16; chunks 18..23 stored fp8 as
            # DoubleRow pairs for the mm2 sprinkle.
            hT = hpool.tile([P, MH16, TB], bf16)
            hT8 = hpool.tile([P, NPR, 2, TB], fp8)
            for mh in range(MH):
                ps = ps1.tile([P, TB], f32)
                for ko in range(KO):
                    lhsT = (pre0_sb[:, ko, TB:] if mh == 0
                            else w1_sb[:, mh, ko])
                    rhs = (pre0_sb[:, ko, :TB] if blk == 0
                           else xe_sb[:, blk, ko])
                    nc.tensor.matmul(ps[:], lhsT, rhs,
                                     start=(ko == 0), stop=(ko == KO - 1))
                if mh < MH16:
                    nc.scalar.activation(hT[:, mh], ps[:], GELU,
                                         bias=b1_sb[:, mh:mh + 1])
                else:
                    pr, j = divmod(mh - MH16, 2)
                    nc.scalar.activation(hT8[:, pr, j], ps[:], GELU,
                                         bias=b1_sb[:, mh:mh + 1])
            # mm2: yT[d, t] = (sum_kh 256*w2[kh,:].T @ h[kh,:]) / 256 + b2
            yT = ypool.tile([P, KO, TB], bf16)
            # Each bf16<->DoubleRow mode transition on the tensor engine
            # costs a ~190ns bubble, so the DR matmuls are clustered: the
            # bf16 parts of three d-chunks run back-to-back, then all
            # three d-chunks' DR matmuls (one transition pair per THREE
            # d-chunks). All three PSUM groups stay open across the
            # cluster (3 single-buf ps2 tags).
            for d0 in range(0, KO, 3):
                pca = ps2.tile([P, TB], f32)
                pcb = ps2.tile([P, TB], f32)
                pcc = ps2.tile([P, TB], f32)
                trio = ((pca, d0), (pcb, d0 + 1), (pcc, d0 + 2))
                for ps, d in trio:
                    for kh in range(MH16):
                        nc.tensor.matmul(ps[:], w2b_sb[:, d, kh],
                                         hT[:, kh], start=(kh == 0),
                                         stop=False)
                last_trio = blk == NBLK - 1 and d0 == KO - 3
                for ps, d in trio:
                    for pr in range(NPR):
                        nc.tensor.matmul(ps[:], w28_sb[:, d, pr],
                                         hT8[:, pr], start=False,
                                         stop=(pr == NPR - 1),
                                         perf_mode=DR)
                for ps, d in trio:
                    if last_trio and d >= KO - 2:
                        # Tail: both final d-chunks' bias-activations run
                        # split across the scalar and vector engines, and
                        # the output stores issue on both HWDGE rings, so
                        # the post-stream exposure is one short act + one
                        # DMA receipt.
                        hh = TB // 2
                        nc.scalar.activation(yT[:, d, :hh], ps[:, :hh],
                                             IDENT, bias=b2_sb[:, d:d + 1],
                                             scale=1.0 / W2S)
                        nc.vector.tensor_scalar(
                            yT[:, d, hh:], ps[:, hh:], 1.0 / W2S,
                            b2_sb[:, d:d + 1], mybir.AluOpType.mult,
                            mybir.AluOpType.add)
                        nc.sync.dma_start(out[:, blk, d, :hh],
                                          yT[:, d, :hh])
                        nc.scalar.dma_start(out[:, blk, d, hh:],
                                            yT[:, d, hh:])
                    else:
                        nc.scalar.activation(yT[:, d], ps[:], IDENT,
                                             bias=b2_sb[:, d:d + 1],
                                             scale=1.0 / W2S)
                        nc.sync.dma_start(out[:, blk, d], yT[:, d])

    nc.compile()
    return nc


def _route(x_flat, Wr):
    """Reproduce the reference router exactly: softmax -> top-2 -> renormalize
    -> capacity-limited keep in flat (token-major, k-inner) order."""
    logits = x_flat @ Wr
    m = logits.max(-1, keepdims=True)
    ex = np.exp(logits - m)
    probs = ex / ex.sum(-1, keepdims=True)
    n = np.arange(N_TOK)
    i1 = probs.argmax(-1)
    p1 = probs[n, i1]
    probs2 = probs.copy()
    probs2[n, i1] = -np.inf
    i2 = probs2.argmax(-1)
    p2 = probs[n, i2]
    s = p1 + p2
    e_flat = np.stack([i1, i2], -1).reshape(-1)          # [2N] expert ids
    p_flat = np.stack([p1 / s, p2 / s], -1).reshape(-1)  # [2N] combine weights
    order = np.argsort(e_flat, kind="stable")            # flat order per expert
    sorted_e = e_flat[order]
    starts = np.searchsorted(sorted_e, np.arange(E))
    ends = np.searchsorted(sorted_e, np.arange(E), side="right")
    toks, wgts = [], []
    for e in range(E):
        kept = order[starts[e] : min(ends[e], starts[e] + CAP)]
        toks.append(kept // TOPK)
        wgts.append(p_flat[kept].astype(np.float32))
    return toks, wgts


def kernel(x, Wr, w1, b1, w2, b2):
    _ensure_ntff_hook_importable()
    from concourse import bass_utils

    x = np.asarray(x, np.float32)
    Wr = np.asarray(Wr, np.float32)
    w1 = np.asarray(w1, np.float32)
    b1 = np.asarray(b1, np.float32)
    w2 = np.asarray(w2, np.float32)
    b2 = np.asarray(b2, np.float32)

    x_flat = x.reshape(N_TOK, D)
    toks, wgts = _route(x_flat, Wr)

    if "nc" not in _CACHE:
        _CACHE["nc"] = _build_nc()
    nc = _CACHE["nc"]

    bf = ml_dtypes.bfloat16
    f8 = ml_dtypes.float8_e4m3fn
    in_maps = []
    for e in range(E):
        cnt = len(toks[e])
        xe = np.zeros((CAP, D), np.float32)
        xe[:cnt] = x_flat[toks[e]]
        xeT = np.ascontiguousarray(
            xe.reshape(NBLK, TB, KO, P).transpose(3, 0, 2, 1)).astype(bf)
        w1r = np.ascontiguousarray(
            w1[e].reshape(KO, P, MH, P).transpose(1, 2, 0, 3)).astype(bf)
        w2s = (w2[e] * W2S).reshape(MH, P, KO, P)
        # [pi, d, kh, di] from w2s[kh, pi, d, di]
        w2r = w2s.transpose(1, 2, 0, 3)
        w2b_ = np.ascontiguousarray(w2r[:, :, :MH16]).astype(bf)
        w28_ = np.ascontiguousarray(
            np.clip(w2r[:, :, MH16:], -240.0, 240.0)
        ).reshape(P, KO, NPR, 2, P).astype(f8)
        in_maps.append({
            "pre0": np.ascontiguousarray(
                np.concatenate([xeT[:, 0], w1r[:, 0]], axis=-1)),
            "xeT": xeT,
            "w1": w1r,
            "b1": np.ascontiguousarray(b1[e].reshape(MH, P).T),
            "w2b": w2b_,
            "w28": w28_,
            "b2": np.ascontiguousarray(b2[e].reshape(KO, P).T),
        })

    trace = bool(os.environ.get("MOE_TRACE"))
    try:
        res = bass_utils.run_bass_kernel_spmd(
            nc, in_maps, core_ids=list(range(NCORES)), trace=trace)
    except Exception:
        if trace or os.environ.get("BASS_TRACE"):
            # Profiling infrastructure failure — rerun without tracing.
            os.environ["BASS_NEVER_TRACE"] = "1"
            res = bass_utils.run_bass_kernel_spmd(
                nc, in_maps, core_ids=list(range(NCORES)), trace=False)
        else:
            raise
    _CACHE["last_results"] = res

    out = np.zeros((N_TOK, D), np.float32)
    for e in range(E):
        y = res.results[e]["out"].astype(np.float32)   # [P, NBLK, KO, TB]
        y = y.transpose(1, 3, 2, 0).reshape(CAP, D)
        cnt = len(toks[e])
        # token ids are unique within one expert, so fancy-index += is safe
        out[toks[e]] += y[:cnt] * wgts[e][:, None]
    return out.reshape(B, S, D)
